# revision 47
# baseline (speedup 1.0000x reference)
"""BDH dense-transformer Trainium2 kernel (8 NeuronCores, SPMD).

Model (weight-tied, 4 layers): T=1024, D=256, NH=4, N=8192/head, VOCAB=256.

Sharding: core c -> head h=c//2, latent half j=c%2 (4096 latent dims/core).
  - encoder/encoder_v column-sharded, decoder row-sharded (host-permuted so
    rope pairs are de-interleaved: local m in [0,2048) = even pair elements,
    [2048,4096) = odd elements; permutation applied consistently to all three
    weight shards so scores/gate/decoder are unaffected).
  - scores trick: yKV = mask(qr qr^T) @ x distributes over latent shards:
    each core computes mask(qr_loc qr_loc^T) @ x; the pair exchanges yKV
    partials with a pairwise AllGather (bf16) and sums locally.
  - decoder partial sums: 8-way AllReduce on y (bf16), LN/residual fully
    local; the LAST layer instead does one ReduceScatter and each core
    emits logits for its own 128-token chunk (host reassembles).

qr is stored fp8e4 scaled by SC_SCALE (scale folded into the resident fp8
cos/sin tables; drains unscale via activation scale / scaled mask), and the
score matmuls run in fp8 with 128-col causal trimming of each strip.
kmode="plain" (default) uses per-chunk fp8 matmuls; kmode="dr"
(DoubleRow, KMODE=dr) is numerically correct via the start=False zero-base
idiom but measured ~10% slower end-to-end (LDWEIGHTS economics) — see
memory trn2-doublerow-fp8-broken / trn2-bdh-perf-findings. NOTE: the Tile
scheduler re-derives instruction order from deps + its own (collective-
pessimistic) cost model, so emission-order/priority tweaks do not change
the schedule.

Layer is a software pipeline over two 512-token blocks (tb=0,1):
  B(tb):   x_sp(tb)=relu(W_e^T xT(tb)), rope -> qr(tb), spill x_sp; score
           tiles whose rhs lives in tb accumulate lagged inside the loop
           (sc0 = scores cols tb0, si 0..3; sc1a = scores cols tb1, si 0..3).
           Score strips are causally trimmed at 128-col granularity.
  sc1b:    scores cols tb1, si 4..7 (needs qr(tb1) rows).
  yKV(tb): masked scores @ x -> pairwise AllGather + local add (AR1).
  F(tb):   y_sp=relu(W_v^T yKVlnT), gate with reloaded x_sp, decoder partial.
  ARy(tb): 8-way AllReduce of partial y (bf16), then local LN+residual+LN.
  xupd(tb) feeds the next layer's B(tb); mask0/yKV0/AR1(0) are emitted before
  xupd1(L-1) so AR1(0) is never gated by the previous layer's tb1 LN chain.
Collectives (and the DMAs feeding/draining them) are the only thing on the
Pool queue; PSUM->SBUF casts ride the Activation engine (Copy) to unload DVE.

On-device layouts: big tensors feature-major [latent, T] (contraction dims on
partitions); residual stream x token-major [T, D] fp32; matmuls bf16 with
fp32 PSUM accumulation.
"""

import os
import numpy as np
import ml_dtypes

BF16NP = ml_dtypes.bfloat16
F8NP = ml_dtypes.float8_e4m3

# full-size config
T = 1024
D = 256
NH = 4
N = 8192
V = 256
L = 4
NCORES = 8
P = 128
EPS = 1e-5
THETA = 2.0 ** 16
# qr is stored fp8e4 scaled by SC_SCALE (folded into the host cs tables);
# score drains unscale by SC_INV (folded into mask_sb for diagonal blocks)
SC_SCALE = 64.0
SC_INV = 1.0 / (SC_SCALE * SC_SCALE)

_CACHE = {}


def _cfg(n_cores=NCORES, half=N // 2, t=T, layers=L, no_cc=False,
         kmode=os.environ.get("KMODE", "plain")):
    # half: latent width per core (= N*NH/n_cores)
    assert t % 512 == 0 and half % 256 == 0
    return dict(
        n_cores=n_cores, half=half, t=t, layers=layers, no_cc=no_cc, kmode=kmode,
        tch=t // P,            # token chunks
        kch=half // P,         # latent chunks
        pblk=half // 2 // P,   # pair blocks (even/odd chunk pairs)
        tb_n=t // 512,         # 512-wide t blocks
        dch=D // P,            # 2
    )


# ---------------------------------------------------------------- device code

def emit_model(tc, in_aps, out_ap, cfg):
    from contextlib import ExitStack
    import concourse.mybir as mybir
    from concourse.masks import make_identity

    nc = tc.nc
    f32 = mybir.dt.float32
    bf = mybir.dt.bfloat16
    f8 = mybir.dt.float8e4
    DRMODE = mybir.MatmulPerfMode.DoubleRow
    ADD = mybir.AluOpType.add
    SUB = mybir.AluOpType.subtract
    MULT = mybir.AluOpType.mult
    Relu = mybir.ActivationFunctionType.Relu
    Sqrt = mybir.ActivationFunctionType.Sqrt
    Copy = mybir.ActivationFunctionType.Copy

    n_cores = cfg["n_cores"]
    TCH, KCH, PBLK, TB, DCH = (cfg[k] for k in ("tch", "kch", "pblk", "tb_n", "dch"))
    TT = cfg["t"]
    HALF = cfg["half"]
    layers = cfg["layers"]
    no_cc = cfg.get("no_cc", False)
    pair_groups = [[2 * i, 2 * i + 1] for i in range(max(n_cores // 2, 1))]
    all_group = [list(range(n_cores))]
    NSB = 4 * TB          # score row strips for the last t-block

    with ExitStack() as ctx:
        singles = ctx.enter_context(tc.tile_pool(name="singles", bufs=1))
        state = ctx.enter_context(tc.tile_pool(name="state", bufs=1))
        dram = ctx.enter_context(tc.tile_pool(name="dramp", bufs=1, space="DRAM"))
        lnp = ctx.enter_context(tc.tile_pool(name="lnp", bufs=4))

        # ---- resident constants (x0 is DMA'd first, below; w_e next: B0
        # depends only on those, and the x0 LN chain overlaps the w_e DMA)
        we_sb = singles.tile([P, DCH, HALF], bf, name="we_sb")
        nc.sync.dma_start(out=we_sb[:], in_=in_aps["w_e"].rearrange("(ko p) m -> p ko m", p=P))
        # resident rope tables: planes [cos*S, sin*S] in fp8 (32KB/partition)
        cs_sb = singles.tile([P, PBLK, 2, TT], f8, name="cs_sb")
        nc.sync.dma_start(out=cs_sb[:],
                          in_=in_aps["cs_t"].rearrange("(b p) c t -> p b c t", p=P))
        mask_sb = singles.tile([P, P], f32, name="mask_sb")
        nc.gpsimd.dma_start(out=mask_sb[:], in_=in_aps["mask"][:])
        wv_sb = singles.tile([P, DCH, HALF], bf, name="wv_sb")
        nc.gpsimd.dma_start(out=wv_sb[:], in_=in_aps["w_v"].rearrange("(ko p) m -> p ko m", p=P))
        dec_sb = singles.tile([P, KCH, D], bf, name="dec_sb")
        nc.gpsimd.dma_start(out=dec_sb[:], in_=in_aps["dec"].rearrange("(kc p) d -> p kc d", p=P))
        lm_sb = singles.tile([P, DCH, V], bf, name="lm_sb")
        nc.gpsimd.dma_start(out=lm_sb[:], in_=in_aps["lm"].rearrange("(ko p) v -> p ko v", p=P))
        eps_sb = singles.tile([P, 1], f32, name="eps_sb")
        nc.vector.memset(eps_sb[:], EPS)
        id_bf = singles.tile([P, P], bf, name="id_bf")
        make_identity(nc, id_bf[:])

        # ---- resident state
        x_sb = state.tile([P, TCH, D], f32, name="x_sb")        # residual, token-major
        tmaj_bf = state.tile([P, TCH, D], bf, name="tmaj_bf")   # bf16 copy of x
        ykvln_bf = state.tile([P, TCH, D], bf, name="ykvln_bf")  # LN(yKV) bf16
        dmaj_bf = state.tile([P, DCH, TT], bf, name="dmaj_bf")  # xT / yKV_lnT
        qr_sb = state.tile([P, KCH, TT], f8, name="qr_sb")
        td_f32 = state.tile([P, TCH, D], f32, name="td_f32")    # x0 / LN scratch
        yt_bf = state.tile([P, DCH, TT], bf, name="yt_bf")      # partial y feature-major
        # collective staging (token-major bf16)
        yk_stage = [state.tile([P, 4, D], bf, name=f"yk_stage{tb}") for tb in range(TB)]
        ykv_post = [state.tile([P, 4, D], bf, name=f"ykv_post{tb}") for tb in range(TB)]
        y_stage = [state.tile([P, 4, D], bf, name=f"y_stage{tb}") for tb in range(TB)]
        y_post = [state.tile([P, 4, D], bf, name=f"y_post{tb}") for tb in range(TB)]

        # ---- dram scratch
        xsp_dram = dram.tile([P, PBLK, 2, TT], bf, name="xsp_dram")
        ar1_ins = [dram.tile([P, 4, D], bf, name=f"ar1_in{tb}") for tb in range(TB)]
        ar1_outs = [dram.tile([2, P, 4, D], bf, name=f"ar1_out{tb}") for tb in range(TB)]
        ary_ins = [dram.tile([P, 4, D], bf, name=f"ary_in{tb}") for tb in range(TB)]
        ary_mids = [dram.tile([P * 4 * D // max(n_cores, 1)], bf,
                              name=f"ary_mid{tb}") for tb in range(TB)]
        ary_outs = [dram.tile([P, 4, D], bf, name=f"ary_out{tb}") for tb in range(TB)]
        rs_in = dram.tile([TCH, P, D], bf, name="rs_in")
        rs_out = dram.tile([1, P, D], bf, name="rs_out")

        def act_copy(out, in_):
            nc.scalar.activation(out=out, in_=in_, func=Copy)

        def emit_ln_2d(src2, dst2):
            # LayerNorm over D of one [P, D] tile
            stats = lnp.tile([P, 6], f32, name="ln_stats", tag="ln_stats")
            nc.vector.bn_stats(out=stats[:], in_=src2)
            mv = lnp.tile([P, 2], f32, name="ln_mv", tag="ln_mv")
            nc.vector.bn_aggr(out=mv[:], in_=stats[:])
            std = lnp.tile([P, 1], f32, name="ln_std", tag="ln_std")
            nc.scalar.activation(out=std[:], in_=mv[:, 1:2], func=Sqrt,
                                 bias=eps_sb[:, 0:1])
            rstd = lnp.tile([P, 1], f32, name="ln_rstd", tag="ln_rstd")
            nc.vector.reciprocal(out=rstd[:], in_=std[:])
            nc.vector.tensor_scalar(out=dst2, in0=src2,
                                    scalar1=mv[:, 0:1], scalar2=rstd[:],
                                    op0=SUB, op1=MULT)

        def emit_ln_grp(src3, dsts):
            # 4 independent LayerNorms over D (HW BNStats requires exactly 6
            # outputs/partition, so stats stay per-tau)
            for sub in range(4):
                emit_ln_2d(src3[:, sub, :], dsts[sub])

        def cc(kind, op, groups, in_t, out_t):
            if no_cc:
                nc.gpsimd.dma_start(out=out_t[:], in_=in_t[:])
            else:
                nc.gpsimd.collective_compute(
                    kind, op, replica_groups=groups,
                    ins=[in_t.opt()], outs=[out_t.opt()])

        def emit_transpose(ps_pool, src2, dst2, tag):
            pt = ps_pool.tile([P, P], bf, name=f"tp_{tag}", tag=f"tp_{tag}")
            nc.tensor.transpose(pt[:], src2, id_bf[:])
            act_copy(dst2, pt[:])

        def emit_transpose2(ps_pool, srcs, dst, tag):
            # transpose two 128x128 blocks into one 2-slot PSUM tile and
            # drain with a single wide ACT copy (GPSIMD cannot access PSUM;
            # DMA XBAR transposes measured ~+300us end-to-end — do not use)
            pt = ps_pool.tile([P, 2, P], bf, name=f"tp2_{tag}", tag=f"tp2_{tag}")
            for k, src in enumerate(srcs):
                nc.tensor.transpose(pt[:, k, :], src, id_bf[:])
            act_copy(dst, pt[:])

        def emit_B(layer, tb, work, xspP, emit_sc):
            """x_sp/rope/spill for t-block tb; emit_sc(blk) emits the lagged
            score accumulation whose rhs lives in this t-block."""
            LAG = 2
            c0, c1 = tb * 512, (tb + 1) * 512

            for blk in range(PBLK + LAG):
                if blk < PBLK:
                    vb = work.tile([P, 2, 512], bf, name="vb", tag="vb")
                    for parity in (0, 1):
                        kc = blk + PBLK * parity
                        pt = xspP.tile([P, 512], f32, name="xsp_ps", tag="xsp_ps")
                        for ko in range(DCH):
                            nc.tensor.matmul(
                                pt[:], lhsT=we_sb[:, ko, kc * P:(kc + 1) * P],
                                rhs=dmaj_bf[:, ko, c0:c1],
                                start=(ko == 0), stop=(ko == DCH - 1))
                        nc.scalar.activation(out=vb[:, parity, :], in_=pt[:], func=Relu)
                    nc.sync.dma_start(out=xsp_dram[:, blk, :, c0:c1], in_=vb[:])
                    # rope (DVE; Pool stays reserved for collectives —
                    # measured slower when the odd-output chain rode Pool).
                    # Resident cs planes are [cos, sin] so the odd-output
                    # products are two single-plane multiplies.
                    csb = cs_sb[:, blk, :, c0:c1]
                    t12 = work.tile([P, 2, 512], bf, name="t12", tag="t12", bufs=8)
                    nc.vector.tensor_mul(out=t12[:], in0=vb[:], in1=csb)
                    nc.vector.tensor_tensor(qr_sb[:, blk, c0:c1],
                                            t12[:, 0, :], t12[:, 1, :], SUB)
                    t34 = work.tile([P, 2, 512], bf, name="t34", tag="t12", bufs=8)
                    nc.vector.tensor_mul(out=t34[:, 0, :], in0=vb[:, 0, :],
                                         in1=csb[:, 1, :])
                    nc.vector.tensor_mul(out=t34[:, 1, :], in0=vb[:, 1, :],
                                         in1=csb[:, 0, :])
                    nc.vector.tensor_tensor(qr_sb[:, blk + PBLK, c0:c1],
                                            t34[:, 0, :], t34[:, 1, :], ADD)
                if blk >= LAG:
                    emit_sc(blk - LAG)

        # Score strips: strip si of t-block tbc covers cols [rel_lo, 512)
        # (128-granularity causal trim; rel_lo>0 only when the diagonal block
        # sits inside this t-block). Scores accumulate in fp8 DoubleRow mode
        # over adjacent latent-chunk pairs (2 k-tiles per instruction); the
        # PSUM holds SC_SCALE^2 * score and drains apply SC_INV.
        def strip_rel(si, tbc):
            return max(si * P - tbc * 512, 0)

        def strip_pieces(si, tbc):
            # <=256-col pieces (DoubleRow moving limit)
            out = []
            start = strip_rel(si, tbc)
            while start < 512:
                w = min(256, 512 - start)
                out.append((start, w))
                start += w
            return out

        def make_strip_ps(pool, pfx, tbc, si_list):
            return {si: pool.tile([P, 512 - strip_rel(si, tbc)], f32,
                                  name=f"{pfx}_{si}", tag=f"{pfx}_{si}", bufs=1)
                    for si in si_list}

        kmode = cfg.get("kmode", "dr")

        def emit_sc_pair(sc_ps, si_list, tbc, a, which="all"):
            # One chunk-pair (a, a+1) of score matmuls. A DoubleRow start=True
            # resets the has-written state of the WHOLE PSUM tile (HW-bisected),
            # so only the FIRST piece's first pair carries start=True; the other
            # pieces never use start — their first write lands on the reset
            # has-written bits, which gives direct-write (zero-base) semantics
            # (validated on HW, probe H5/H6). All pieces interleave lagged.
            for si in si_list:
                rel_lo = strip_rel(si, tbc)
                if kmode == "plain":
                    if which == "rest":
                        continue
                    for kc in (a, a + 1):
                        nc.tensor.matmul(
                            sc_ps[si][:, 0:512 - rel_lo],
                            lhsT=qr_sb[:, kc, si * P:(si + 1) * P],
                            rhs=qr_sb[:, kc, tbc * 512 + rel_lo:(tbc + 1) * 512],
                            start=(a == 0 and kc == a),
                            stop=(a == KCH - 2 and kc == a + 1),
                            skip_group_check=True)
                    continue
                if which == "rest":
                    continue
                pieces = strip_pieces(si, tbc)
                for pi, (st, w) in enumerate(pieces):
                    nc.tensor.matmul(
                        sc_ps[si][:, st - rel_lo:st - rel_lo + w],
                        lhsT=qr_sb[:, a:a + 2, si * P:(si + 1) * P],
                        rhs=qr_sb[:, a:a + 2, tbc * 512 + st:tbc * 512 + st + w],
                        start=(a == 0 and pi == 0),
                        stop=(a == KCH - 2 and pi == len(pieces) - 1),
                        perf_mode=DRMODE, skip_group_check=True)

        def emit_sc_rest(sc_ps, si_list, tbc):
            # no tails needed anymore (kept as a no-op hook; plain mode and the
            # start=False piece scheme both emit everything in the lag)
            for si in si_list:
                for m in range(KCH // 2):
                    emit_sc_pair(sc_ps, (si,), tbc, 2 * m, which="rest")

        def drain_strip(sc_ps, si, tbc, sc_sb):
            # mask (diag block) / scaled-copy one strip into sc_sb row si
            rel_lo = strip_rel(si, tbc)
            pt = sc_ps[si]
            if si >= 4 * tbc:
                nc.vector.tensor_mul(out=sc_sb[:, si, rel_lo:rel_lo + P],
                                     in0=pt[:, 0:P], in1=mask_sb[:])
                if rel_lo + P < 512:
                    nc.scalar.activation(out=sc_sb[:, si, rel_lo + P:512],
                                         in_=pt[:, P:], func=Copy, scale=SC_INV)
            else:
                nc.scalar.activation(out=sc_sb[:, si, :], in_=pt[:],
                                     func=Copy, scale=SC_INV)

        def emit_ykv(tb, sc_sb, ykP):
            # yKV partials for taus of tb -> yk_stage[tb] (bf16)
            for sub in range(4):
                tau = tb * 4 + sub
                yk = ykP.tile([P, D], f32, name="yk_ps", tag="yk_ps")
                for si in range(tau + 1):
                    nc.tensor.matmul(
                        yk[:], lhsT=sc_sb[:, si, sub * P:(sub + 1) * P],
                        rhs=tmaj_bf[:, si, :],
                        start=(si == 0), stop=(si == tau))
                act_copy(yk_stage[tb][:, sub, :], yk[:])

        def emit_ar1(tb):
            # pairwise AllGather of the two yKV partials (cheaper than
            # AllReduce); the add happens locally on DVE before LN
            nc.gpsimd.dma_start(out=ar1_ins[tb][:], in_=yk_stage[tb][:])
            if n_cores > 1 and not no_cc:
                nc.gpsimd.collective_compute(
                    "AllGather", mybir.AluOpType.bypass,
                    replica_groups=pair_groups,
                    ins=[ar1_ins[tb].opt()], outs=[ar1_outs[tb].opt()])
            else:
                for r in range(2):
                    nc.gpsimd.dma_start(out=ar1_outs[tb][r:r + 1],
                                        in_=ar1_ins[tb][:])

        def emit_ln_etp(tb, tpps):
            # LN(yKV) for taus of tb, then transpose into dmaj columns of tb
            pair_sb = lnp.tile([P, 2, 4, D], bf, name=f"ykv_pair{tb}",
                               tag=f"ykv_pair{tb}", bufs=1)
            nc.sync.dma_start(out=pair_sb[:],
                              in_=ar1_outs[tb].rearrange("r p c d -> p r c d"))
            nc.vector.tensor_tensor(ykv_post[tb][:], pair_sb[:, 0],
                                    pair_sb[:, 1], ADD)
            emit_ln_grp(ykv_post[tb][:],
                        [ykvln_bf[:, tb * 4 + s, :] for s in range(4)])
            for sub in range(4):
                tau = tb * 4 + sub
                emit_transpose2(
                    tpps, [ykvln_bf[:, tau, ko * P:(ko + 1) * P] for ko in range(DCH)],
                    dmaj_bf[:, :, tau * P:(tau + 1) * P], "e")

        def emit_F(layer, tb, work, psY, psF, tpps):
            c0, c1 = tb * 512, (tb + 1) * 512
            yT_ps = [psY.tile([P, 512], f32, name=f"yt_ps{dh}", tag=f"yt_ps{dh}")
                     for dh in range(DCH)]
            for blk in range(PBLK):
                xsp = work.tile([P, 2, 512], bf, name="xsp_r", tag="xsp_r")
                nc.sync.dma_start(out=xsp[:], in_=xsp_dram[:, blk, :, c0:c1])
                for parity in (0, 1):
                    kc = blk + PBLK * parity
                    ysp = work.tile([P, 512], bf, name="ysp", tag="ysp", bufs=6)
                    pt = psF.tile([P, 512], f32, name="ysp_ps", tag="ysp_ps")
                    for ko in range(DCH):
                        nc.tensor.matmul(
                            pt[:], lhsT=wv_sb[:, ko, kc * P:(kc + 1) * P],
                            rhs=dmaj_bf[:, ko, c0:c1],
                            start=(ko == 0), stop=(ko == DCH - 1))
                    nc.scalar.activation(out=ysp[:], in_=pt[:], func=Relu)
                    xy = work.tile([P, 512], bf, name="xy", tag="xy", bufs=6)
                    nc.vector.tensor_mul(out=xy[:], in0=xsp[:, parity, :], in1=ysp[:])
                    first = (blk == 0 and parity == 0)
                    last = (blk == PBLK - 1 and parity == 1)
                    for dh in range(DCH):
                        nc.tensor.matmul(
                            yT_ps[dh][:],
                            lhsT=dec_sb[:, kc, dh * P:(dh + 1) * P],
                            rhs=xy[:],
                            start=first, stop=last, skip_group_check=True)
            for dh in range(DCH):
                act_copy(yt_bf[:, dh, c0:c1], yT_ps[dh][:])
            # transpose partial y to token-major bf16 stage
            for sub in range(4):
                tau = tb * 4 + sub
                emit_transpose2(
                    tpps, [yt_bf[:, dh, tau * P:(tau + 1) * P] for dh in range(DCH)],
                    y_stage[tb][:, sub, :], "g")

        def emit_ary(tb):
            # single-launch 8-way AllReduce (RS+AG split is cheaper in the sim
            # cost model but measurably slower on hardware)
            nc.gpsimd.dma_start(out=ary_ins[tb][:], in_=y_stage[tb][:])
            if n_cores > 1:
                cc("AllReduce", ADD, all_group, ary_ins[tb], ary_outs[tb])
            else:
                nc.gpsimd.dma_start(out=ary_outs[tb][:], in_=ary_ins[tb][:])
            nc.gpsimd.dma_start(out=y_post[tb][:], in_=ary_outs[tb][:])

        def emit_xupd(tb, tpps, tag="a"):
            # x = LN(x + LN(y)) for taus of tb; refresh tmaj/dmaj
            t0, t1 = tb * 4, (tb + 1) * 4
            emit_ln_grp(y_post[tb][:], [td_f32[:, t0 + s, :] for s in range(4)])
            nc.vector.tensor_tensor(td_f32[:, t0:t1, :], td_f32[:, t0:t1, :],
                                    x_sb[:, t0:t1, :], ADD)
            emit_ln_grp(td_f32[:, t0:t1, :], [x_sb[:, t0 + s, :] for s in range(4)])
            for sub in range(4):
                tau = t0 + sub
                act_copy(tmaj_bf[:, tau, :], x_sb[:, tau, :])
                emit_transpose2(
                    tpps, [tmaj_bf[:, tau, ko * P:(ko + 1) * P] for ko in range(DCH)],
                    dmaj_bf[:, :, tau * P:(tau + 1) * P], tag)

        out_r = out_ap.rearrange("(tau p) v -> p tau v", p=P)

        def emit_final(pid):
            # last layer: single ReduceScatter of the full y; each core LNs,
            # adds its own residual chunk, and emits logits for its own 128
            # tokens into output rows [0:P] (host reassembles across cores).
            from concourse.bass import ds
            for tb in range(TB):
                nc.gpsimd.dma_start(
                    out=rs_in[tb * 4:(tb + 1) * 4].rearrange("c p d -> p c d"),
                    in_=y_stage[tb][:])
            if n_cores > 1 and not no_cc:
                nc.gpsimd.collective_compute(
                    "ReduceScatter", ADD, replica_groups=all_group,
                    ins=[rs_in.opt()], outs=[rs_out.opt()])
            else:
                nc.gpsimd.dma_start(out=rs_out[:], in_=rs_in[0:1])
            with tc.tile_pool(name="fin", bufs=1) as fw, \
                 tc.tile_pool(name="finps", bufs=2, space="PSUM") as ps:
                ych = fw.tile([P, 1, D], bf, name="ych")
                nc.gpsimd.dma_start(out=ych[:], in_=rs_out.rearrange("c p d -> p c d"))
                yln = fw.tile([P, 1, D], f32, name="yln")
                emit_ln_2d(ych[:, 0, :], yln[:, 0, :])
                if pid is not None:
                    x_dyn = x_sb[:, ds(pid, 1), :]
                else:
                    x_dyn = x_sb[:, 0:1, :]
                nc.vector.tensor_tensor(yln[:], yln[:], x_dyn, ADD)
                xf = fw.tile([P, D], bf, name="xf")
                emit_ln_2d(yln[:, 0, :], xf[:])
                xfT = fw.tile([P, D], bf, name="xfT")
                for ko in range(DCH):
                    emit_transpose(ps, xf[:, ko * P:(ko + 1) * P],
                                   xfT[:, ko * P:(ko + 1) * P], "z")
                pt = ps.tile([P, V], f32, name="lg_ps", tag="lg_ps")
                for ko in range(DCH):
                    nc.tensor.matmul(pt[:], lhsT=xfT[:, ko * P:(ko + 1) * P],
                                     rhs=lm_sb[:, ko, :], start=(ko == 0),
                                     stop=(ko == DCH - 1))
                lg = fw.tile([P, V], f32, name="lg_sb")
                nc.vector.tensor_copy(out=lg[:], in_=pt[:])
                nc.sync.dma_start(out=out_r[:, 0, :], in_=lg[:])

        # ---- initial: x = LN(x0); tmaj/dmaj per t-block
        pid = nc.vector.partition_id() if n_cores > 1 else None
        nc.scalar.dma_start(out=td_f32[:], in_=in_aps["x0"].rearrange("(tau p) d -> p tau d", p=P))
        with tc.tile_pool(name="tpI", bufs=2, space="PSUM") as tpps:
            for tau in range(TCH):
                emit_ln_2d(td_f32[:, tau, :], x_sb[:, tau, :])
            nc.vector.tensor_copy(out=tmaj_bf[:], in_=x_sb[:])
            for tau in range(TCH):
                emit_transpose2(
                    tpps, [tmaj_bf[:, tau, ko * P:(ko + 1) * P] for ko in range(DCH)],
                    dmaj_bf[:, :, tau * P:(tau + 1) * P], "i")

        for layer in range(layers):
            last = layer == layers - 1
            scw_cm = tc.tile_pool(name=f"scb_{layer}", bufs=1)
            scw = scw_cm.__enter__()
            # ---------------- B0 (+ lagged trimmed sc0)
            sc0_cm = tc.tile_pool(name=f"sc0_{layer}", bufs=1, space="PSUM")
            sc0 = sc0_cm.__enter__()
            sc0_ps = make_strip_ps(sc0, "sc0", 0, range(4))

            def emit_sc0(blk):
                if kmode == "drbatch":
                    return
                if blk % 2 == 1:
                    emit_sc_pair(sc0_ps, range(4), 0, blk - 1, "first")
                    emit_sc_pair(sc0_ps, range(4), 0, blk - 1 + PBLK, "first")

            with tc.tile_pool(name=f"b0_{layer}", bufs=3) as work, \
                 tc.tile_pool(name=f"b0ps{layer}", bufs=3, space="PSUM") as xspP:
                emit_B(layer, 0, work, xspP, emit_sc0)
            if kmode == "drbatch":
                for m in range(KCH // 2):
                    emit_sc_pair(sc0_ps, range(4), 0, 2 * m, "first")
            emit_sc_rest(sc0_ps, range(4), 0)

            # ---------------- mask0 + yKV0 + AR1(0)  (before xupd1 so AR1(0)
            # is never gated by the previous layer's tb1 LN chain)
            sc_sb0 = scw.tile([P, 4, 512], bf, name="sc_sb0", tag="sc_sb0")
            for si in range(4):
                drain_strip(sc0_ps, si, 0, sc_sb0)
            sc0_cm.__exit__(None, None, None)
            with tc.tile_pool(name=f"yk0_{layer}", bufs=2, space="PSUM") as ykP:
                emit_ykv(0, sc_sb0, ykP)
            emit_ar1(0)

            # ---------------- xupd1 of previous layer (feeds B1)
            if layer > 0:
                with tc.tile_pool(name=f"tpU1_{layer}", bufs=2, space="PSUM") as tpps:
                    emit_xupd(1, tpps)

            # ---------------- B1 (+ lagged sc1a si0..3 and strips si4/si5);
            # trimmed strips si6/si7 after; mask1 ; yKV1
            sc1bA_cm = tc.tile_pool(name=f"sc1bA_{layer}", bufs=1, space="PSUM")
            sc1bA = sc1bA_cm.__enter__()
            sc1bA_ps = make_strip_ps(sc1bA, "sc1bA", 1, (4, 5))
            sc1a_cm = tc.tile_pool(name=f"sc1a_{layer}", bufs=1, space="PSUM")
            sc1a = sc1a_cm.__enter__()
            sc1a_ps = make_strip_ps(sc1a, "sc1a", 1, range(4))
            sc1a_ps.update(sc1bA_ps)

            def emit_sc1_lag(blk):
                if kmode == "drbatch":
                    return
                if blk % 2 == 1:
                    emit_sc_pair(sc1a_ps, range(6), 1, blk - 1, "first")
                    emit_sc_pair(sc1a_ps, range(6), 1, blk - 1 + PBLK, "first")

            with tc.tile_pool(name=f"b1_{layer}", bufs=3) as work, \
                 tc.tile_pool(name=f"b1ps{layer}", bufs=2, space="PSUM") as xspP:
                emit_B(layer, 1, work, xspP, emit_sc1_lag)
            if kmode == "drbatch":
                for m in range(KCH // 2):
                    emit_sc_pair(sc1a_ps, range(6), 1, 2 * m, "first")
            emit_sc_rest(sc1a_ps, range(6), 1)
            sc_sb1 = scw.tile([P, 8, 512], bf, name="sc_sb1", tag="sc_sb1")
            # sc1a/si4/si5 strips copy+mask while si6/si7 compute
            for si in range(4):
                drain_strip(sc1a_ps, si, 1, sc_sb1)
            sc1a_cm.__exit__(None, None, None)
            for si in (4, 5):
                drain_strip(sc1a_ps, si, 1, sc_sb1)
            sc1bB_cm = tc.tile_pool(name=f"sc1bB_{layer}", bufs=1, space="PSUM")
            sc1bB = sc1bB_cm.__enter__()
            sc1bB_ps = make_strip_ps(sc1bB, "sc1bB", 1, (6, 7))
            for m in range(KCH // 2):
                emit_sc_pair(sc1bB_ps, (6, 7), 1, 2 * m)
            for si in (6, 7):
                drain_strip(sc1bB_ps, si, 1, sc_sb1)
            with tc.tile_pool(name=f"yk1_{layer}", bufs=2, space="PSUM") as ykP:
                emit_ykv(1, sc_sb1, ykP)
            sc1bB_cm.__exit__(None, None, None)
            sc1bA_cm.__exit__(None, None, None)
            emit_ar1(1)

            # ---------------- LN0/Etp0 -> F0 -> ARy(0)
            with tc.tile_pool(name=f"e0_{layer}", bufs=2, space="PSUM") as tpE, \
                 tc.tile_pool(name=f"f0_{layer}", bufs=4) as work, \
                 tc.tile_pool(name=f"f0y{layer}", bufs=1, space="PSUM") as psY, \
                 tc.tile_pool(name=f"f0s{layer}", bufs=2, space="PSUM") as psF:
                emit_ln_etp(0, tpE)
                emit_F(layer, 0, work, psY, psF, tpE)
                if not last:
                    emit_ary(0)

            # ---------------- LN1/Etp1 -> F1 -> ARy(1)
            with tc.tile_pool(name=f"e1_{layer}", bufs=2, space="PSUM") as tpE, \
                 tc.tile_pool(name=f"f1_{layer}", bufs=4) as work, \
                 tc.tile_pool(name=f"f1y{layer}", bufs=1, space="PSUM") as psY, \
                 tc.tile_pool(name=f"f1s{layer}", bufs=2, space="PSUM") as psF:
                emit_ln_etp(1, tpE)
                emit_F(layer, 1, work, psY, psF, tpE)
                if not last:
                    emit_ary(1)

            if last:
                emit_final(pid)
            else:
                # ---------------- xupd0 (tb0 residual update; feeds next B0)
                # NOTE: emitting this earlier (inside F1, even with
                # tc.high_priority) is a scheduling no-op — the Tile scheduler
                # re-derives the same schedule because its cost model says
                # ARy(0) completes late; the emission position is irrelevant.
                with tc.tile_pool(name=f"tpU0_{layer}", bufs=2, space="PSUM") as tpps:
                    emit_xupd(0, tpps)
            scw_cm.__exit__(None, None, None)


def build(cfg):
    import concourse.bacc as bacc
    import concourse.tile as tile
    import concourse.mybir as mybir

    f32 = mybir.dt.float32
    bf = mybir.dt.bfloat16
    nc = bacc.Bacc("TRN2", target_bir_lowering=False, debug=False,
                   enable_asserts=False, num_devices=cfg["n_cores"])
    TT, HALF = cfg["t"], cfg["half"]
    in_aps = {
        "x0": nc.dram_tensor("x0", [TT, D], f32, kind="ExternalInput").ap(),
        "w_e": nc.dram_tensor("w_e", [D, HALF], bf, kind="ExternalInput").ap(),
        "w_v": nc.dram_tensor("w_v", [D, HALF], bf, kind="ExternalInput").ap(),
        "dec": nc.dram_tensor("dec", [HALF, D], bf, kind="ExternalInput").ap(),
        "lm": nc.dram_tensor("lm", [D, V], bf, kind="ExternalInput").ap(),
        "cs_t": nc.dram_tensor("cs_t", [HALF // 2, 2, TT], mybir.dt.float8e4,
                               kind="ExternalInput").ap(),
        "mask": nc.dram_tensor("mask", [P, P], f32, kind="ExternalInput").ap(),
    }
    out_ap = nc.dram_tensor("logits", [TT, V], f32, kind="ExternalOutput").ap()
    with tile.TileContext(nc) as tc:
        emit_model(tc, in_aps, out_ap, cfg)
    nc.compile()
    return nc


# ---------------------------------------------------------------- host side

def make_tables(t, n_full):
    # mirror the reference fp32 math
    n = np.arange(n_full, dtype=np.float32)
    q = np.floor(n / 2.0).astype(np.float32) * np.float32(2.0)
    base = np.power(np.float32(THETA), (q / np.float32(n_full)).astype(np.float32))
    freqs = (np.float32(1.0) / base / np.float32(2.0 * np.pi)).astype(np.float32)
    tt = np.arange(t, dtype=np.float32)[:, None]
    phases = (tt * freqs[None, :]).astype(np.float32)
    ph = ((phases % np.float32(1.0)) * np.float32(2.0 * np.pi)).astype(np.float32)
    return np.cos(ph).astype(np.float32), np.sin(ph).astype(np.float32)


def make_in_maps(idx, embed, encoder, encoder_v, decoder, lm_head, cfg):
    n_cores = cfg["n_cores"]
    half = cfg["half"]
    t = cfg["t"]
    ph_loc = half // 2
    nh = max(n_cores // 2, 1)
    n_full = half * 2  # per-head latent dim

    idx = np.asarray(idx).astype(np.int64)
    embed = np.asarray(embed, dtype=np.float32)
    enc = np.asarray(encoder, dtype=np.float32)
    enc_v = np.asarray(encoder_v, dtype=np.float32)
    dec = np.asarray(decoder, dtype=np.float32).reshape(nh, n_full, D)
    lm = np.asarray(lm_head, dtype=np.float32)

    x0 = embed[idx[0]].astype(np.float32)               # [t, D]
    cos_f, sin_f = make_tables(t, n_full)               # [t, n_full]
    cos_f = cos_f * np.float32(SC_SCALE)
    sin_f = sin_f * np.float32(SC_SCALE)
    lm_bf = lm.astype(BF16NP)
    mask = (np.arange(P)[:, None] < np.arange(P)[None, :]).astype(np.float32) \
        * np.float32(SC_INV)

    in_maps = []
    for c in range(n_cores):
        h, j = divmod(c, 2)
        p_glob = j * ph_loc + np.arange(ph_loc)
        cols = np.concatenate([2 * p_glob, 2 * p_glob + 1])
        in_maps.append({
            "x0": x0,
            "w_e": np.ascontiguousarray(enc[h][:, cols]).astype(BF16NP),
            "w_v": np.ascontiguousarray(enc_v[h][:, cols]).astype(BF16NP),
            "dec": np.ascontiguousarray(dec[h][cols, :]).astype(BF16NP),
            "lm": lm_bf,
            "cs_t": np.ascontiguousarray(np.stack(
                [cos_f[:, 2 * p_glob].T, sin_f[:, 2 * p_glob].T],
                axis=1)).astype(BF16NP).astype(F8NP),
            "mask": mask,
        })
    return in_maps


def _get_nc(cfg_key=None, cfg=None):
    if cfg is None:
        cfg = _cfg()
    key = tuple(sorted(cfg.items()))
    if key not in _CACHE:
        _CACHE[key] = build(cfg)
    return _CACHE[key]


def run(inputs, cfg=None, trace=False, **run_kwargs):
    from concourse.bass_utils import run_bass_kernel_spmd
    if cfg is None:
        cfg = _cfg()
    nc = _get_nc(cfg=cfg)
    in_maps = make_in_maps(inputs["idx"], inputs["embed"], inputs["encoder"],
                           inputs["encoder_v"], inputs["decoder"],
                           inputs["lm_head"], cfg)
    res = run_bass_kernel_spmd(nc, in_maps, core_ids=list(range(cfg["n_cores"])),
                               trace=trace, **run_kwargs)
    # each core writes logits for its own 128-token chunk into rows [0:P]
    logits = np.concatenate(
        [np.asarray(res.results[c]["logits"][:P], dtype=np.float32)
         for c in range(cfg["n_cores"])], axis=0)
    return logits.reshape(1, cfg["t"], V), res


def kernel(idx, embed, encoder, encoder_v, decoder, lm_head):
    logits, _ = run(dict(idx=idx, embed=embed, encoder=encoder,
                         encoder_v=encoder_v, decoder=decoder, lm_head=lm_head))
    return logits

